# revision 12
# baseline (speedup 1.0000x reference)
"""HDSharedExpertBasis kernel for 8 Trainium2 NeuronCores.

Strategy: shard both shared-basis tensors over the hd_dim axis (each core
owns a 512-wide slice of hd_dim).  All device traffic is fp16 (tolerance
2e-2; fp16 keeps 10 mantissa bits so basis quantization + 32-term
accumulation error is ~0.2%), which halves HBM traffic vs fp32.

Per core:
    W_up_c  = sum_v cu[v] * B_up[v][:, c*512:(c+1)*512]
    W_dn_c  = sum_v cd[v] * B_dn[v][c*512:(c+1)*512, :]
    hiddenT = gelu(W_up_c^T @ X^T)       [512, 8192] fp16, kept in SBUF
    partial = hiddenT^T @ W_dn_c         [8192, 1024] fp16 partial -> HBM
Host sums the 8 fp16 partials in fp32 and adds the expert bias.

Engine assignment (from measured rates: ACT activation and DVE
scalar_tensor_tensor run 1x for fp16; DVE tensor_tensor/tensor_scalar get
2x/4x packed modes):
  - PE builds W_up while the bu stream arrives: matmul with stationary
    c_v*I accumulates sum_v c_v*B_v directly in PSUM (all 8 banks), using
    otherwise-idle PE time before mm1 can start.  ~107us DMA-paced.
  - DVE builds W_dn during mm1 (tensor_scalar mult + tensor_tensor add
    per jh half-tile) - DVE is otherwise idle then.
  - ACT does the 64 gelus (mm1) + 8 W_up PSUM casts + 128 mm2 PSUM->SBUF
    fp16 copies (mm2 phase, otherwise idle).
  - gpsimd queue streams xt, paced by pool slots; sync queue streams
    bu -> bd -> out writes (batched 512KB).
mm2 runs in two d-column-half passes so the first half starts as soon as
the first half of W_dn is built.
"""

import sys

sys.path.insert(0, "/opt/trn_rl_repo")

import numpy as np

N_TOK = 8192
D = 1024
HD = 4096
V = 32
NCORES = 8
HSH = HD // NCORES          # 512 hd slice per core
TOKCH = 512                 # token chunk (matmul moving dim)
NCH = N_TOK // TOKCH        # 16
KD = D // 128               # 8  k-chunks for matmul 1
KH = HSH // 128             # 4  k-chunks for matmul 2
VB = 2                      # bu basis vectors per DMA batch
VBD = 4                     # bd basis vectors per DMA batch

_CACHE = {}
LAST_RESULT = None          # BassKernelResults of the most recent run


def _build():
    if "nc" in _CACHE:
        return _CACHE["nc"]
    import concourse.mybir as mybir
    import concourse.tile as tile
    from concourse import bacc

    f16 = mybir.dt.float16
    f32 = mybir.dt.float32
    GELU = mybir.ActivationFunctionType.Gelu
    COPY = mybir.ActivationFunctionType.Copy
    MUL = mybir.AluOpType.mult
    ADD = mybir.AluOpType.add

    nc = bacc.Bacc(None, target_bir_lowering=False)
    # Host pre-arranged layouts (see kernel()):
    xt = nc.dram_tensor("xt", [NCH, 128, KD, TOKCH], f16, kind="ExternalInput")
    bu = nc.dram_tensor("bu", [V // VB, 128, VB, KD, HSH], f16, kind="ExternalInput")
    bd = nc.dram_tensor("bd", [2, V // VBD, 128, VBD, KH, 512], f16, kind="ExternalInput")
    ident = nc.dram_tensor("ident", [128, 128], f16, kind="ExternalInput")
    cu = nc.dram_tensor("cu", [128, V], f32, kind="ExternalInput")
    cd = nc.dram_tensor("cd", [128, V], f32, kind="ExternalInput")
    out = nc.dram_tensor("out", [N_TOK, D], f16, kind="ExternalOutput")

    with tile.TileContext(nc) as tc:
        with (
            tc.tile_pool(name="wpool", bufs=1) as wpool,
            tc.tile_pool(name="bupool", bufs=2) as bupool,
            tc.tile_pool(name="bdpool", bufs=2) as bdpool,
            tc.tile_pool(name="tmppool", bufs=2) as tmppool,
            tc.tile_pool(name="xtpool", bufs=4) as xtpool,
            tc.tile_pool(name="hgpool", bufs=64) as hgpool,
            tc.tile_pool(name="opool", bufs=2) as opool,
            tc.tile_pool(name="scpool", bufs=2) as scpool,
            tc.tile_pool(name="pall", bufs=8, space="PSUM") as pall,
        ):
            id_t = wpool.tile([128, 128], f16)
            cu_t = wpool.tile([128, V], f32)
            cd_t = wpool.tile([128, V], f32)
            nc.scalar.dma_start(id_t[:], ident[:])
            nc.scalar.dma_start(cu_t[:], cu[:])
            nc.scalar.dma_start(cd_t[:], cd[:])

            # ---- xt stream on gpsimd queue, paced by pool slots ----
            # (model-time floor keeps it from stealing bu bandwidth early)
            xt_tiles = {}
            with tc.tile_wait_until(0.075):
                for ch in range(NCH):
                    t = xtpool.tile([128, KD, TOKCH], f16, tag="xt")
                    nc.gpsimd.dma_start(t[:], xt[ch, :, :, :])
                    xt_tiles[ch] = t

            # ---- W_up build on PE: psum[kd] += (c_v I)^T @ B_v[:, kd, :] ----
            # All 8 PSUM banks hold W_up in fp32 during the bu stream.
            ps_w = [
                pall.tile([128, 512], f32, name="ps", tag="ps") for j in range(8)
            ]
            for vb in range(V // VB):
                bu_t = bupool.tile([128, VB, KD, HSH], f16, tag="bu")
                nc.sync.dma_start(bu_t[:], bu[vb, :, :, :, :])
                for i in range(VB):
                    v = vb * VB + i
                    sc_id = scpool.tile([128, 128], f16, tag="sc")
                    nc.vector.tensor_scalar(
                        sc_id[:], id_t[:], cu_t[:, v : v + 1], None, MUL
                    )
                    for kd in range(KD):
                        nc.tensor.matmul(
                            ps_w[kd][:],
                            sc_id[:],
                            bu_t[:, i, kd, :],
                            start=(v == 0),
                            stop=(v == V - 1),
                        )
            w_up = wpool.tile([128, KD, HSH], f16)
            for kd in range(KD):
                nc.scalar.activation(w_up[:, kd, :], ps_w[kd][:], COPY)

            # ---- W_down build on DVE (scale 4x + add 2x), two jh halves ----
            w_dn = wpool.tile([128, KH, D], f16)
            ctx_bd = tc.tile_wait_until(0.09)
            ctx_bd.__enter__()
            for jh in range(2):
                sl = slice(jh * 512, (jh + 1) * 512)
                for vb in range(V // VBD):
                    bd_t = bdpool.tile([128, VBD, KH, 512], f16, tag="bd")
                    nc.sync.dma_start(bd_t[:], bd[jh, vb, :, :, :, :])
                    for i in range(VBD):
                        v = vb * VBD + i
                        if v == 0:
                            nc.vector.tensor_scalar(
                                w_dn[:, :, sl], bd_t[:, i], cd_t[:, 0:1],
                                None, MUL,
                            )
                        else:
                            tmp = tmppool.tile([128, KH, 512], f16, tag="tmpd")
                            nc.vector.tensor_scalar(
                                tmp[:], bd_t[:, i], cd_t[:, v : v + 1],
                                None, MUL,
                            )
                            nc.vector.tensor_tensor(
                                w_dn[:, :, sl], w_dn[:, :, sl], tmp[:], ADD
                            )

            ctx_bd.__exit__(None, None, None)

            # ---- mm1 + gelu over all token chunks; hidden kept in SBUF ----
            hg = {}
            for ch in range(NCH):
                xt_t = xt_tiles.pop(ch)
                for ht in range(4):
                    ph = pall.tile([128, TOKCH], f32, name="ps", tag="ps")
                    for kd in range(KD):
                        nc.tensor.matmul(
                            ph[:],
                            w_up[:, kd, ht * 128 : (ht + 1) * 128],
                            xt_t[:, kd, :],
                            start=(kd == 0),
                            stop=(kd == KD - 1),
                        )
                    g = hgpool.tile([128, TOKCH], f16, tag="hg")
                    nc.scalar.activation(g[:], ph[:], GELU)
                    hg[(ch, ht)] = g

            # ---- mm2 in two column-half passes (jn matches bd jh) ----
            for jn in range(2):
                for ch in range(NCH):
                    o_sb = opool.tile([128, 4, 512], f16, tag="o", name="o_sb")
                    for ts in range(4):
                        po = pall.tile([128, 512], f32, name="ps", tag="ps")
                        for kh in range(KH):
                            nc.tensor.matmul(
                                po[:],
                                hg[(ch, kh)][:, ts * 128 : (ts + 1) * 128],
                                w_dn[:, kh, jn * 512 : (jn + 1) * 512],
                                start=(kh == 0),
                                stop=(kh == KH - 1),
                            )
                        nc.scalar.activation(o_sb[:, ts, :], po[:], COPY)
                    dst = out[
                        ch * TOKCH : (ch + 1) * TOKCH,
                        jn * 512 : (jn + 1) * 512,
                    ].rearrange("(ts p) j -> p ts j", p=128)
                    nc.scalar.dma_start(dst, o_sb[:])

    nc.compile()
    _CACHE["nc"] = nc
    return nc


def _prep_inputs(
    inputs, shared_basis_up, shared_basis_down, expert_coeffs_up,
    expert_coeffs_down, expert_idx,
):
    """Host-side layout/dtype prep: fp16 conversion + per-core tiling."""
    idx = int(np.asarray(expert_idx))
    cu = np.ascontiguousarray(
        np.broadcast_to(
            np.asarray(expert_coeffs_up, np.float32)[idx][None, :], (128, V)
        )
    )
    ident = np.ascontiguousarray(np.eye(128, dtype=np.float16))
    cd = np.ascontiguousarray(
        np.broadcast_to(
            np.asarray(expert_coeffs_down, np.float32)[idx][None, :], (128, V)
        )
    )
    # xt[ch, p, kd, t] = X[ch*512 + t, kd*128 + p]
    x16 = np.asarray(inputs, np.float16)
    xt = np.ascontiguousarray(
        x16.reshape(NCH, TOKCH, KD, 128).transpose(0, 3, 2, 1)
    )
    sbu = np.asarray(shared_basis_up, np.float16)
    sbd = np.asarray(shared_basis_down, np.float16)

    in_maps = []
    for c in range(NCORES):
        # bu[vb, p, i, kd, h] = B_up[v=vb*VB+i, kd*128 + p, c*512 + h]
        bu_c = np.ascontiguousarray(
            sbu[:, :, c * HSH : (c + 1) * HSH]
            .reshape(V // VB, VB, KD, 128, HSH)
            .transpose(0, 3, 1, 2, 4)
        )
        # bd[jh, vb, p, i, kh, j] = B_dn[v, c*512 + kh*128 + p, jh*512 + j]
        bd_c = np.ascontiguousarray(
            sbd[:, c * HSH : (c + 1) * HSH, :]
            .reshape(V // VBD, VBD, KH, 128, 2, 512)
            .transpose(4, 0, 3, 1, 2, 5)
        )
        in_maps.append({"xt": xt, "bu": bu_c, "bd": bd_c, "ident": ident,
                        "cu": cu, "cd": cd})
    return in_maps


def kernel(
    inputs,
    shared_basis_up,
    shared_basis_down,
    expert_coeffs_up,
    expert_coeffs_down,
    expert_bias,
    expert_idx,
    _trace=False,
):
    global LAST_RESULT
    from concourse import bass_utils

    nc = _build()
    in_maps = _prep_inputs(
        inputs, shared_basis_up, shared_basis_down, expert_coeffs_up,
        expert_coeffs_down, expert_idx,
    )

    res = bass_utils.run_bass_kernel_spmd(
        nc,
        in_maps,
        core_ids=list(range(NCORES)),
        trace=_trace,
        trace_cores=list(range(NCORES)) if _trace else None,
    )
    LAST_RESULT = res

    idx = int(np.asarray(expert_idx))
    total = res.results[0]["out"].astype(np.float32)
    for c in range(1, NCORES):
        total += res.results[c]["out"].astype(np.float32)
    total += np.asarray(expert_bias, np.float32)[idx][None, :]
    return total


# revision 13
# speedup vs baseline: 1.0356x; 1.0356x over previous
"""HDSharedExpertBasis kernel for 8 Trainium2 NeuronCores.

Strategy: shard both shared-basis tensors over the hd_dim axis (each core
owns a 512-wide slice of hd_dim).  All device traffic is fp16 (tolerance
2e-2; fp16 keeps 10 mantissa bits so basis quantization + 32-term
accumulation error is ~0.2%), which halves HBM traffic vs fp32.

Per core:
    W_up_c  = sum_v cu[v] * B_up[v][:, c*512:(c+1)*512]
    W_dn_c  = sum_v cd[v] * B_dn[v][c*512:(c+1)*512, :]
    hiddenT = gelu(W_up_c^T @ X^T)       [512, 8192] fp16, kept in SBUF
    partial = hiddenT^T @ W_dn_c         [8192, 1024] fp16 partial -> HBM
Host sums the 8 fp16 partials in fp32 and adds the expert bias.

Engine assignment (from measured rates: ACT activation and DVE
scalar_tensor_tensor run 1x for fp16; DVE tensor_tensor/tensor_scalar get
2x/4x packed modes):
  - PE builds W_up while the bu stream arrives: matmul with stationary
    c_v*I accumulates sum_v c_v*B_v directly in PSUM (all 8 banks), using
    otherwise-idle PE time before mm1 can start.  ~107us DMA-paced.
  - DVE builds W_dn during mm1 (tensor_scalar mult + tensor_tensor add
    per jh half-tile) - DVE is otherwise idle then.
  - ACT does the 64 gelus (mm1) + 8 W_up PSUM casts + 128 mm2 PSUM->SBUF
    fp16 copies (mm2 phase, otherwise idle).
  - gpsimd queue streams xt, paced by pool slots; sync queue streams
    bu -> bd -> out writes (batched 512KB).
mm2 runs in two d-column-half passes so the first half starts as soon as
the first half of W_dn is built.
"""

import sys

sys.path.insert(0, "/opt/trn_rl_repo")

import numpy as np

N_TOK = 8192
D = 1024
HD = 4096
V = 32
NCORES = 8
HSH = HD // NCORES          # 512 hd slice per core
TOKCH = 512                 # token chunk (matmul moving dim)
NCH = N_TOK // TOKCH        # 16
KD = D // 128               # 8  k-chunks for matmul 1
KH = HSH // 128             # 4  k-chunks for matmul 2
VB = 2                      # bu basis vectors per DMA batch
VBD = 4                     # bd basis vectors per DMA batch

_CACHE = {}
LAST_RESULT = None          # BassKernelResults of the most recent run


def _build():
    if "nc" in _CACHE:
        return _CACHE["nc"]
    import concourse.mybir as mybir
    import concourse.tile as tile
    from concourse import bacc

    f16 = mybir.dt.float16
    f32 = mybir.dt.float32
    GELU = mybir.ActivationFunctionType.Gelu
    COPY = mybir.ActivationFunctionType.Copy
    MUL = mybir.AluOpType.mult
    ADD = mybir.AluOpType.add

    nc = bacc.Bacc(None, target_bir_lowering=False)
    # Host pre-arranged layouts (see kernel()):
    xt = nc.dram_tensor("xt", [NCH, 128, KD, TOKCH], f16, kind="ExternalInput")
    bu = nc.dram_tensor("bu", [V // VB, 128, VB, KD, HSH], f16, kind="ExternalInput")
    bd = nc.dram_tensor("bd", [2, V // VBD, 128, VBD, KH, 512], f16, kind="ExternalInput")
    ident = nc.dram_tensor("ident", [128, 128], f16, kind="ExternalInput")
    cu = nc.dram_tensor("cu", [128, V], f32, kind="ExternalInput")
    cd = nc.dram_tensor("cd", [128, V], f32, kind="ExternalInput")
    out = nc.dram_tensor("out", [N_TOK, D], f16, kind="ExternalOutput")

    with tile.TileContext(nc) as tc:
        with (
            tc.tile_pool(name="wpool", bufs=1) as wpool,
            tc.tile_pool(name="bupool", bufs=3) as bupool,
            tc.tile_pool(name="bdpool", bufs=2) as bdpool,
            tc.tile_pool(name="tmppool", bufs=2) as tmppool,
            tc.tile_pool(name="xtpool", bufs=3) as xtpool,
            tc.tile_pool(name="hgpool", bufs=64) as hgpool,
            tc.tile_pool(name="opool", bufs=2) as opool,
            tc.tile_pool(name="scpool", bufs=2) as scpool,
            tc.tile_pool(name="pall", bufs=8, space="PSUM") as pall,
        ):
            id_t = wpool.tile([128, 128], f16)
            cu_t = wpool.tile([128, V], f32)
            cd_t = wpool.tile([128, V], f32)
            nc.scalar.dma_start(id_t[:], ident[:])
            nc.scalar.dma_start(cu_t[:], cu[:])
            nc.scalar.dma_start(cd_t[:], cd[:])

            # ---- xt stream on gpsimd queue, paced by pool slots ----
            # (model-time floor keeps it from stealing bu bandwidth early)
            xt_tiles = {}
            for ch in range(NCH):
                t = xtpool.tile([128, KD, TOKCH], f16, tag="xt")
                nc.gpsimd.dma_start(t[:], xt[ch, :, :, :])
                xt_tiles[ch] = t

            # ---- W_up build on PE: psum[kd] += (c_v I)^T @ B_v[:, kd, :] ----
            # All 8 PSUM banks hold W_up in fp32 during the bu stream.
            ps_w = [
                pall.tile([128, 512], f32, name="ps", tag="ps") for j in range(8)
            ]
            for vb in range(V // VB):
                bu_t = bupool.tile([128, VB, KD, HSH], f16, tag="bu")
                nc.sync.dma_start(bu_t[:], bu[vb, :, :, :, :])
                for i in range(VB):
                    v = vb * VB + i
                    sc_id = scpool.tile([128, 128], f16, tag="sc")
                    nc.vector.tensor_scalar(
                        sc_id[:], id_t[:], cu_t[:, v : v + 1], None, MUL
                    )
                    for kd in range(KD):
                        nc.tensor.matmul(
                            ps_w[kd][:],
                            sc_id[:],
                            bu_t[:, i, kd, :],
                            start=(v == 0),
                            stop=(v == V - 1),
                        )
            w_up = wpool.tile([128, KD, HSH], f16)
            for kd in range(KD):
                nc.scalar.activation(w_up[:, kd, :], ps_w[kd][:], COPY)

            # ---- W_down build on DVE (scale 4x + add 2x), two jh halves ----
            w_dn = wpool.tile([128, KH, D], f16)
            for jh in range(2):
                sl = slice(jh * 512, (jh + 1) * 512)
                for vb in range(V // VBD):
                    bd_t = bdpool.tile([128, VBD, KH, 512], f16, tag="bd")
                    nc.sync.dma_start(bd_t[:], bd[jh, vb, :, :, :, :])
                    for i in range(VBD):
                        v = vb * VBD + i
                        if v == 0:
                            nc.vector.tensor_scalar(
                                w_dn[:, :, sl], bd_t[:, i], cd_t[:, 0:1],
                                None, MUL,
                            )
                        else:
                            tmp = tmppool.tile([128, KH, 512], f16, tag="tmpd")
                            nc.vector.tensor_scalar(
                                tmp[:], bd_t[:, i], cd_t[:, v : v + 1],
                                None, MUL,
                            )
                            nc.vector.tensor_tensor(
                                w_dn[:, :, sl], w_dn[:, :, sl], tmp[:], ADD
                            )

            # ---- mm1 + gelu over all token chunks; hidden kept in SBUF ----
            hg = {}
            for ch in range(NCH):
                xt_t = xt_tiles.pop(ch)
                for ht in range(4):
                    ph = pall.tile([128, TOKCH], f32, name="ps", tag="ps")
                    for kd in range(KD):
                        nc.tensor.matmul(
                            ph[:],
                            w_up[:, kd, ht * 128 : (ht + 1) * 128],
                            xt_t[:, kd, :],
                            start=(kd == 0),
                            stop=(kd == KD - 1),
                        )
                    g = hgpool.tile([128, TOKCH], f16, tag="hg")
                    nc.scalar.activation(g[:], ph[:], GELU)
                    hg[(ch, ht)] = g

            # ---- mm2 in two column-half passes (jn matches bd jh) ----
            for jn in range(2):
                for ch in range(NCH):
                    o_sb = opool.tile([128, 4, 512], f16, tag="o", name="o_sb")
                    for ts in range(4):
                        po = pall.tile([128, 512], f32, name="ps", tag="ps")
                        for kh in range(KH):
                            nc.tensor.matmul(
                                po[:],
                                hg[(ch, kh)][:, ts * 128 : (ts + 1) * 128],
                                w_dn[:, kh, jn * 512 : (jn + 1) * 512],
                                start=(kh == 0),
                                stop=(kh == KH - 1),
                            )
                        nc.scalar.activation(o_sb[:, ts, :], po[:], COPY)
                    dst = out[
                        ch * TOKCH : (ch + 1) * TOKCH,
                        jn * 512 : (jn + 1) * 512,
                    ].rearrange("(ts p) j -> p ts j", p=128)
                    nc.scalar.dma_start(dst, o_sb[:])

    nc.compile()
    _CACHE["nc"] = nc
    return nc


def _prep_inputs(
    inputs, shared_basis_up, shared_basis_down, expert_coeffs_up,
    expert_coeffs_down, expert_idx,
):
    """Host-side layout/dtype prep: fp16 conversion + per-core tiling."""
    idx = int(np.asarray(expert_idx))
    cu = np.ascontiguousarray(
        np.broadcast_to(
            np.asarray(expert_coeffs_up, np.float32)[idx][None, :], (128, V)
        )
    )
    ident = np.ascontiguousarray(np.eye(128, dtype=np.float16))
    cd = np.ascontiguousarray(
        np.broadcast_to(
            np.asarray(expert_coeffs_down, np.float32)[idx][None, :], (128, V)
        )
    )
    # xt[ch, p, kd, t] = X[ch*512 + t, kd*128 + p]
    x16 = np.asarray(inputs, np.float16)
    xt = np.ascontiguousarray(
        x16.reshape(NCH, TOKCH, KD, 128).transpose(0, 3, 2, 1)
    )
    sbu = np.asarray(shared_basis_up, np.float16)
    sbd = np.asarray(shared_basis_down, np.float16)

    in_maps = []
    for c in range(NCORES):
        # bu[vb, p, i, kd, h] = B_up[v=vb*VB+i, kd*128 + p, c*512 + h]
        bu_c = np.ascontiguousarray(
            sbu[:, :, c * HSH : (c + 1) * HSH]
            .reshape(V // VB, VB, KD, 128, HSH)
            .transpose(0, 3, 1, 2, 4)
        )
        # bd[jh, vb, p, i, kh, j] = B_dn[v, c*512 + kh*128 + p, jh*512 + j]
        bd_c = np.ascontiguousarray(
            sbd[:, c * HSH : (c + 1) * HSH, :]
            .reshape(V // VBD, VBD, KH, 128, 2, 512)
            .transpose(4, 0, 3, 1, 2, 5)
        )
        in_maps.append({"xt": xt, "bu": bu_c, "bd": bd_c, "ident": ident,
                        "cu": cu, "cd": cd})
    return in_maps


def kernel(
    inputs,
    shared_basis_up,
    shared_basis_down,
    expert_coeffs_up,
    expert_coeffs_down,
    expert_bias,
    expert_idx,
    _trace=False,
):
    global LAST_RESULT
    from concourse import bass_utils

    nc = _build()
    in_maps = _prep_inputs(
        inputs, shared_basis_up, shared_basis_down, expert_coeffs_up,
        expert_coeffs_down, expert_idx,
    )

    res = bass_utils.run_bass_kernel_spmd(
        nc,
        in_maps,
        core_ids=list(range(NCORES)),
        trace=_trace,
        trace_cores=list(range(NCORES)) if _trace else None,
    )
    LAST_RESULT = res

    idx = int(np.asarray(expert_idx))
    total = res.results[0]["out"].astype(np.float32)
    for c in range(1, NCORES):
        total += res.results[c]["out"].astype(np.float32)
    total += np.asarray(expert_bias, np.float32)[idx][None, :]
    return total


# revision 14
# speedup vs baseline: 1.0430x; 1.0071x over previous
"""HDSharedExpertBasis kernel for 8 Trainium2 NeuronCores.

Strategy: shard both shared-basis tensors over the hd_dim axis (each core
owns a 512-wide slice of hd_dim).  All device traffic is fp16 (tolerance
2e-2; fp16 keeps 10 mantissa bits so basis quantization + 32-term
accumulation error is ~0.2%), which halves HBM traffic vs fp32.

Per core:
    W_up_c  = sum_v cu[v] * B_up[v][:, c*512:(c+1)*512]
    W_dn_c  = sum_v cd[v] * B_dn[v][c*512:(c+1)*512, :]
    hiddenT = gelu(W_up_c^T @ X^T)       [512, 8192] fp16, kept in SBUF
    partial = hiddenT^T @ W_dn_c         [8192, 1024] fp16 partial -> HBM
Host sums the 8 fp16 partials in fp32 and adds the expert bias.

Engine assignment (from measured rates: ACT activation and DVE
scalar_tensor_tensor run 1x for fp16; DVE tensor_tensor/tensor_scalar get
2x/4x packed modes):
  - PE builds W_up while the bu stream arrives: matmul with stationary
    c_v*I accumulates sum_v c_v*B_v directly in PSUM (all 8 banks), using
    otherwise-idle PE time before mm1 can start.  ~107us DMA-paced.
  - DVE builds W_dn during mm1 (tensor_scalar mult + tensor_tensor add
    per jh half-tile) - DVE is otherwise idle then.
  - ACT does the 64 gelus (mm1) + 8 W_up PSUM casts + 128 mm2 PSUM->SBUF
    fp16 copies (mm2 phase, otherwise idle).
  - gpsimd queue streams xt, paced by pool slots; sync queue streams
    bu (2MB batches) then bd (2MB batches); output writes (batched 512KB
    per chunk-half) go on the scalar queue to keep them off the bd path.
mm2 runs in two d-column-half passes so the first half starts as soon as
the first half of W_dn is built.
"""

import sys

sys.path.insert(0, "/opt/trn_rl_repo")

import numpy as np

N_TOK = 8192
D = 1024
HD = 4096
V = 32
NCORES = 8
HSH = HD // NCORES          # 512 hd slice per core
TOKCH = 512                 # token chunk (matmul moving dim)
NCH = N_TOK // TOKCH        # 16
KD = D // 128               # 8  k-chunks for matmul 1
KH = HSH // 128             # 4  k-chunks for matmul 2
VB = 2                      # bu basis vectors per DMA batch
VBD = 4                     # bd basis vectors per DMA batch

_CACHE = {}
LAST_RESULT = None          # BassKernelResults of the most recent run


def _build():
    if "nc" in _CACHE:
        return _CACHE["nc"]
    import concourse.mybir as mybir
    import concourse.tile as tile
    from concourse import bacc

    f16 = mybir.dt.float16
    f32 = mybir.dt.float32
    GELU = mybir.ActivationFunctionType.Gelu
    COPY = mybir.ActivationFunctionType.Copy
    MUL = mybir.AluOpType.mult
    ADD = mybir.AluOpType.add

    nc = bacc.Bacc(None, target_bir_lowering=False)
    # Host pre-arranged layouts (see kernel()):
    xt = nc.dram_tensor("xt", [NCH, 128, KD, TOKCH], f16, kind="ExternalInput")
    bu = nc.dram_tensor("bu", [V // VB, 128, VB, KD, HSH], f16, kind="ExternalInput")
    bd = nc.dram_tensor("bd", [2, V // VBD, 128, VBD, KH, 512], f16, kind="ExternalInput")
    ident = nc.dram_tensor("ident", [128, 128], f16, kind="ExternalInput")
    cu = nc.dram_tensor("cu", [128, V], f32, kind="ExternalInput")
    cd = nc.dram_tensor("cd", [128, V], f32, kind="ExternalInput")
    out = nc.dram_tensor("out", [N_TOK, D], f16, kind="ExternalOutput")

    with tile.TileContext(nc) as tc:
        with (
            tc.tile_pool(name="wpool", bufs=1) as wpool,
            tc.tile_pool(name="bupool", bufs=3) as bupool,
            tc.tile_pool(name="bdpool", bufs=2) as bdpool,
            tc.tile_pool(name="tmppool", bufs=2) as tmppool,
            tc.tile_pool(name="xtpool", bufs=3) as xtpool,
            tc.tile_pool(name="hgpool", bufs=64) as hgpool,
            tc.tile_pool(name="opool", bufs=2) as opool,
            tc.tile_pool(name="scpool", bufs=2) as scpool,
            tc.tile_pool(name="pall", bufs=8, space="PSUM") as pall,
        ):
            id_t = wpool.tile([128, 128], f16)
            cu_t = wpool.tile([128, V], f32)
            cd_t = wpool.tile([128, V], f32)
            nc.scalar.dma_start(id_t[:], ident[:])
            nc.scalar.dma_start(cu_t[:], cu[:])
            nc.scalar.dma_start(cd_t[:], cd[:])

            # ---- xt stream on gpsimd queue, paced by pool slots ----
            # (model-time floor keeps it from stealing bu bandwidth early)
            xt_tiles = {}
            for ch in range(NCH):
                t = xtpool.tile([128, KD, TOKCH], f16, tag="xt")
                nc.gpsimd.dma_start(t[:], xt[ch, :, :, :])
                xt_tiles[ch] = t

            # ---- W_up build on PE: psum[kd] += (c_v I)^T @ B_v[:, kd, :] ----
            # All 8 PSUM banks hold W_up in fp32 during the bu stream.
            ps_w = [
                pall.tile([128, 512], f32, name="ps", tag="ps") for j in range(8)
            ]
            for vb in range(V // VB):
                bu_t = bupool.tile([128, VB, KD, HSH], f16, tag="bu")
                nc.sync.dma_start(bu_t[:], bu[vb, :, :, :, :])
                for i in range(VB):
                    v = vb * VB + i
                    sc_id = scpool.tile([128, 128], f16, tag="sc")
                    nc.vector.tensor_scalar(
                        sc_id[:], id_t[:], cu_t[:, v : v + 1], None, MUL
                    )
                    for kd in range(KD):
                        nc.tensor.matmul(
                            ps_w[kd][:],
                            sc_id[:],
                            bu_t[:, i, kd, :],
                            start=(v == 0),
                            stop=(v == V - 1),
                        )
            w_up = wpool.tile([128, KD, HSH], f16)
            for kd in range(KD):
                nc.scalar.activation(w_up[:, kd, :], ps_w[kd][:], COPY)

            # ---- W_down build on DVE (scale 4x + add 2x), two jh halves ----
            w_dn = wpool.tile([128, KH, D], f16)
            for jh in range(2):
                sl = slice(jh * 512, (jh + 1) * 512)
                for vb in range(V // VBD):
                    bd_t = bdpool.tile([128, VBD, KH, 512], f16, tag="bd")
                    nc.sync.dma_start(bd_t[:], bd[jh, vb, :, :, :, :])
                    for i in range(VBD):
                        v = vb * VBD + i
                        if v == 0:
                            nc.vector.tensor_scalar(
                                w_dn[:, :, sl], bd_t[:, i], cd_t[:, 0:1],
                                None, MUL,
                            )
                        else:
                            tmp = tmppool.tile([128, KH, 512], f16, tag="tmpd")
                            nc.vector.tensor_scalar(
                                tmp[:], bd_t[:, i], cd_t[:, v : v + 1],
                                None, MUL,
                            )
                            nc.vector.tensor_tensor(
                                w_dn[:, :, sl], w_dn[:, :, sl], tmp[:], ADD
                            )

            # ---- mm1 + gelu over all token chunks; hidden kept in SBUF ----
            hg = {}
            for ch in range(NCH):
                xt_t = xt_tiles.pop(ch)
                for ht in range(4):
                    ph = pall.tile([128, TOKCH], f32, name="ps", tag="ps")
                    for kd in range(KD):
                        nc.tensor.matmul(
                            ph[:],
                            w_up[:, kd, ht * 128 : (ht + 1) * 128],
                            xt_t[:, kd, :],
                            start=(kd == 0),
                            stop=(kd == KD - 1),
                        )
                    g = hgpool.tile([128, TOKCH], f16, tag="hg")
                    nc.scalar.activation(g[:], ph[:], GELU)
                    hg[(ch, ht)] = g

            # ---- mm2 in two column-half passes (jn matches bd jh) ----
            for jn in range(2):
                for ch in range(NCH):
                    o_sb = opool.tile([128, 4, 512], f16, tag="o", name="o_sb")
                    for ts in range(4):
                        po = pall.tile([128, 512], f32, name="ps", tag="ps")
                        for kh in range(KH):
                            nc.tensor.matmul(
                                po[:],
                                hg[(ch, kh)][:, ts * 128 : (ts + 1) * 128],
                                w_dn[:, kh, jn * 512 : (jn + 1) * 512],
                                start=(kh == 0),
                                stop=(kh == KH - 1),
                            )
                        nc.scalar.activation(o_sb[:, ts, :], po[:], COPY)
                    dst = out[
                        ch * TOKCH : (ch + 1) * TOKCH,
                        jn * 512 : (jn + 1) * 512,
                    ].rearrange("(ts p) j -> p ts j", p=128)
                    nc.scalar.dma_start(dst, o_sb[:])

    nc.compile()
    _CACHE["nc"] = nc
    return nc


def _prep_inputs(
    inputs, shared_basis_up, shared_basis_down, expert_coeffs_up,
    expert_coeffs_down, expert_idx,
):
    """Host-side layout/dtype prep: fp16 conversion + per-core tiling."""
    idx = int(np.asarray(expert_idx))
    cu = np.ascontiguousarray(
        np.broadcast_to(
            np.asarray(expert_coeffs_up, np.float32)[idx][None, :], (128, V)
        )
    )
    ident = np.ascontiguousarray(np.eye(128, dtype=np.float16))
    cd = np.ascontiguousarray(
        np.broadcast_to(
            np.asarray(expert_coeffs_down, np.float32)[idx][None, :], (128, V)
        )
    )
    # xt[ch, p, kd, t] = X[ch*512 + t, kd*128 + p]
    x16 = np.asarray(inputs, np.float16)
    xt = np.ascontiguousarray(
        x16.reshape(NCH, TOKCH, KD, 128).transpose(0, 3, 2, 1)
    )
    sbu = np.asarray(shared_basis_up, np.float16)
    sbd = np.asarray(shared_basis_down, np.float16)

    in_maps = []
    for c in range(NCORES):
        # bu[vb, p, i, kd, h] = B_up[v=vb*VB+i, kd*128 + p, c*512 + h]
        bu_c = np.ascontiguousarray(
            sbu[:, :, c * HSH : (c + 1) * HSH]
            .reshape(V // VB, VB, KD, 128, HSH)
            .transpose(0, 3, 1, 2, 4)
        )
        # bd[jh, vb, p, i, kh, j] = B_dn[v, c*512 + kh*128 + p, jh*512 + j]
        bd_c = np.ascontiguousarray(
            sbd[:, c * HSH : (c + 1) * HSH, :]
            .reshape(V // VBD, VBD, KH, 128, 2, 512)
            .transpose(4, 0, 3, 1, 2, 5)
        )
        in_maps.append({"xt": xt, "bu": bu_c, "bd": bd_c, "ident": ident,
                        "cu": cu, "cd": cd})
    return in_maps


def kernel(
    inputs,
    shared_basis_up,
    shared_basis_down,
    expert_coeffs_up,
    expert_coeffs_down,
    expert_bias,
    expert_idx,
    _trace=False,
):
    global LAST_RESULT
    from concourse import bass_utils

    nc = _build()
    in_maps = _prep_inputs(
        inputs, shared_basis_up, shared_basis_down, expert_coeffs_up,
        expert_coeffs_down, expert_idx,
    )

    res = bass_utils.run_bass_kernel_spmd(
        nc,
        in_maps,
        core_ids=list(range(NCORES)),
        trace=_trace,
        trace_cores=list(range(NCORES)) if _trace else None,
    )
    LAST_RESULT = res

    idx = int(np.asarray(expert_idx))
    total = res.results[0]["out"].astype(np.float32)
    for c in range(1, NCORES):
        total += res.results[c]["out"].astype(np.float32)
    total += np.asarray(expert_bias, np.float32)[idx][None, :]
    return total
